# revision 23
# baseline (speedup 1.0000x reference)
"""Trainium2 Bass kernel for causal self-attention with ALiBi + GQA.

Problem: B=2, T=2048, C=2048, 16 q-heads / 4 kv-heads, head_dim=128.
  q = x@q_w.T, k = x@k_w.T, v = x@v_w.T (GQA repeat 4x)
  att = softmax(q k^T/sqrt(d) + causal + alibi); out = (att v) @ o_w.T

Sharding over 8 NeuronCores: core c -> batch c//4, kv-group g=c%4
(q-heads 4g..4g+3, kv-head g).  Each core computes attention for its 4
heads on its batch plus a partial o-projection over its 512 channels;
the host sums the 4 partials per batch.

On-chip design (per core, all matmuls fp16, fp32 accumulate — fp16 is
1 cycle/row on the PE like bf16 but with 4x the mantissa; all values
here are bounded so there is no range risk):
  - x is host-cast to fp16 and host-TRANSPOSED (xT [C,T]) so projection
    moving operands load with plain contiguous DMA.
  - Projections make QT [d,t], KT [d,t] (transposed) and V natural
    [t,d] with a ones-column appended, so the AV matmul also emits the
    softmax denominator for free.
  - Scores are computed transposed sT[k,q] = KTblk.T @ QT (moving free
    dim 512, causally narrowed per diagonal offset); ALiBi + causal
    folded in via host-precomputed additive f32 tiles (DVE) and a
    per-(head,offset) bias in the ACT exp.  No max-subtraction needed:
    scores are small (~N(0,0.8)) and masked entries use -1e9.
  - ALiBi+causal masking uses ONE head-independent relative-position
    table M[pi,oidx,mj] = k-q (exact small ints in f16; causal ->
    -60000) plus per-core [P,1] scalar tables: DVE fused
    scalar_tensor_tensor computes t1 = pss*(SCALE/slope) + (k-q) in
    f16, ACT exp applies scale=slope from an AP.  Storing k-q directly
    (instead of slope-scaled masks) keeps f16 rounding error ~5e-4 of
    the EXPONENT at the entries that matter -- slope-scaled f16 masks
    lose 1e-2 to large-term cancellation.  Far blocks underflow exp to
    0, which is exactly the negligible tail of the softmax.
  - ALiBi band sparsity: the narrowest-band head of each group is
    ordered first (h0/h4/h8/h12, slopes >= 0.011); its score/AV blocks
    with q-k >= 11*128 (softmax mass < e^-14) are skipped.
  - y accumulates un-normalized; delayed normalization via per-row
    reciprocal of the ones-column sums, then PE-transpose -> yT feeds
    the o-projection (psum -> ACT/DVE copy -> fp16 -> DMA out; host
    sums the 4 partials per batch in fp32).
  - Scheduling: a warmup matmul burst ramps the PE p-state; chunk
    j+1's x tiles are prefetched and its per-head Q-projection units
    emitted at attention(j) head boundaries (with prev-chunk
    o-projection t-blocks) BEFORE the yT flush, so the in-order PE
    queue always has fat independent work while the DVE->ACT softmax
    chain drains; head h+1's score matmuls are software-pipelined
    into head h's AV chain; the last chunk's o-projection t-blocks
    interleave into its last head's AV loop (no serial tail); chunk
    0's diagonal mask slice (0.5MB) is DMA'd before the fat weight
    tail so attention(0) never waits on the full 2MB mask tensor.
Measured: ~266-268 us/core on TRN2 (NTFF; occasional +15-50us
outlier runs from device contention), L2 rel err 6.6e-4 vs fp32
reference.  (Baseline of this session: 272-274us, 6.3e-4.)

Rejected directions (measured on this HW): AV with V-stationary (fat
512-wide moving) needs softmax denominators off-PE — GPSIMD add-chains
+ partition_all_reduce are ~4x slower than the cost model (634us total);
XBAR dma_start_transpose from SBUF (direct or DRAM-bounced) returns
wrong data (NaN) in this axon environment; matmul moving free dim is
hard-capped at 512 so wider PSUM tiles are impossible. The ones-column
AV is LDW-conservation-optimal: any orientation computing denominators
on-PE needs the same 544 stationary loads.
"""

import math
import sys
from contextlib import ExitStack

import numpy as np

sys.path.insert(0, "/opt/trn_rl_repo")

import ml_dtypes  # noqa: E402

import concourse.bacc as bacc  # noqa: E402
import concourse.bass as bass  # noqa: E402
import concourse.mybir as mybir  # noqa: E402
import concourse.tile as tile  # noqa: E402

F16 = mybir.dt.float16
F32 = mybir.dt.float32
NP_F16 = np.float16

B, T, C = 2, 2048, 2048
H, HKV, D = 16, 4, 128
P = 128
CH = 512                 # q-chunk (moving free dim)
NCB = C // P             # 16 contraction blocks
NTB = T // P             # 16 t-blocks
NCHK = T // CH           # 4 q-chunks
NQH = 4                  # local q heads per core
SCALE = 1.0 / math.sqrt(D)
MASK_NEG = -60000.0     # f16-representable; exp(scale*(-60000)) == 0


def _alibi_slopes(n):
    start = 2 ** (-(2 ** (-(math.log2(n) - 3))))
    return np.array([start * start**i for i in range(n)], dtype=np.float64)


def build_program():
    """Build the (SPMD-identical) single-core program."""
    nc = bacc.Bacc("TRN2", target_bir_lowering=False, debug=False, num_devices=8)

    xT_ap = nc.dram_tensor("xT", [C, T], F16, kind="ExternalInput").ap()
    qwT_ap = nc.dram_tensor("qwT", [P, NCB, NQH * P], F16, kind="ExternalInput").ap()
    kwT_ap = nc.dram_tensor("kwT", [P, NCB, D], F16, kind="ExternalInput").ap()
    vwT_ap = nc.dram_tensor("vwT", [P, NCB, D], F16, kind="ExternalInput").ap()
    owT_ap = nc.dram_tensor("owT", [P, NQH, C], F16, kind="ExternalInput").ap()
    # M[pi, oidx, mj] = k - q (exact small ints in f16) with causal -60000;
    # head-independent: the slope is applied via per-core scalar tables.
    m_ap = nc.dram_tensor("mrel", [P, 16, CH], F16, kind="ExternalInput").ap()
    sc_ap = nc.dram_tensor("sctbl", [P, NQH, 2], F32, kind="ExternalInput").ap()
    id_ap = nc.dram_tensor("ident", [P, P], F16, kind="ExternalInput").ap()
    out_ap = nc.dram_tensor("out_p", [T, C], F16, kind="ExternalOutput").ap()

    EXP = mybir.ActivationFunctionType.Exp

    with tile.TileContext(nc) as tc, ExitStack() as ctx:
        const = ctx.enter_context(tc.tile_pool(name="const", bufs=1))
        qwT_sb = const.tile([P, NCB, NQH * P], F16, name="qwT_sb")
        kwT_sb = const.tile([P, NCB, D], F16, name="kwT_sb")
        vwT_sb = const.tile([P, NCB, D], F16, name="vwT_sb")
        owT_sb = const.tile([P, NQH, C], F16, name="owT_sb")
        m_sb = const.tile([P, 16, CH], F16, name="m_sb")
        sc_sb = const.tile([P, NQH, 2], F32, name="sc_sb")
        id_sb = const.tile([P, P], F16, name="id_sb")

        wid_sb = const.tile([P, P], F16, name="wid_sb")
        QT_sb = const.tile([P, NQH, T], F16, name="QT_sb")
        KT_sb = const.tile([P, T], F16, name="KT_sb")
        Vaug_sb = const.tile([P, NTB, 132], F16, name="Vaug_sb")
        yT_sb = const.tile([P, NQH, T], F16, name="yT_sb")

        # DMA order tuned so warmup + the first productive matmul (Q-proj
        # head 0 of chunk 0) start as early as possible: tiny identity
        # first (warmup reads it), qwT head-0 slice next, then the chunk-0
        # x tiles (issued by project_x below), then the rest.
        nc.sync.dma_start(id_sb[:], id_ap[:])
        nc.sync.dma_start(qwT_sb[:, :, 0:P], qwT_ap[:, :, 0:P])

        nc.vector.memset(Vaug_sb[:, :, 128:129], 1.0)

        xT_pool = ctx.enter_context(tc.tile_pool(name="xT_pool", bufs=32))
        ps_pool = ctx.enter_context(tc.tile_pool(name="ps_pool", bufs=5, space="PSUM"))
        yps_pool = ctx.enter_context(tc.tile_pool(name="yps_pool", bufs=2, space="PSUM"))
        tp_pool = ctx.enter_context(tc.tile_pool(name="tp_pool", bufs=1, space="PSUM"))
        t1_pool = ctx.enter_context(tc.tile_pool(name="t1_pool", bufs=8))
        exp_pool = ctx.enter_context(tc.tile_pool(name="exp_pool", bufs=34))
        oev_pool = ctx.enter_context(tc.tile_pool(name="oev_pool", bufs=8))
        yn_pool = ctx.enter_context(tc.tile_pool(name="yn_pool", bufs=6))
        rc_pool = ctx.enter_context(tc.tile_pool(name="rc_pool", bufs=6))

        # PE p-state warm-up: dependency-free dummy matmuls ramp the tensor
        # engine to full clock while the first input DMAs land.
        nc.vector.memset(wid_sb[:], 0.0)
        for w in range(40):
            wups = yps_pool.tile([P, 132], F32, name=f"wups_{w}", tag="yps")
            nc.tensor.matmul(
                wups[:, 0:128], lhsT=wid_sb[:], rhs=wid_sb[:], start=True, stop=True
            )

        # ---- Fused per-chunk pipeline: project chunk j, then attention for
        # chunk j (legal because causality means queries in chunk j only
        # attend to keys/values t <= chunk j), then its o-projection.
        # Chunk j+1's x tiles are prefetched and its per-head Q-projection
        # units emitted INSIDE attention(j) at head boundaries: independent
        # fat PE work that absorbs the DVE/ACT softmax-chain latency. ----
        def project_x(j):
            t0 = j * CH
            xts = []
            for cb in range(NCB):
                xt = xT_pool.tile([P, CH], F16, name=f"xt_{j}_{cb}", tag="xt")
                nc.sync.dma_start(xt[:], xT_ap[cb * P : (cb + 1) * P, t0 : t0 + CH])
                xts.append(xt)
            return xts

        def qproj_unit(j, qh, xts):
            t0 = j * CH
            ps = ps_pool.tile([P, CH], F32, name=f"psq_{j}_{qh}", tag="ps")
            for cb in range(NCB):
                nc.tensor.matmul(
                    ps[:],
                    lhsT=qwT_sb[:, cb, qh * P : (qh + 1) * P],
                    rhs=xts[cb][:],
                    start=(cb == 0),
                    stop=(cb == NCB - 1),
                )
            # Scalar-engine evac: frees DVE (mask-adds) and unblocks the
            # PSUM pool faster so downstream matmuls don't stall on buffers.
            nc.scalar.copy(QT_sb[:, qh, t0 : t0 + CH], ps[:])

        def project_k(j, xts):
            t0 = j * CH
            psk = ps_pool.tile([P, CH], F32, name=f"psk_{j}", tag="ps")
            for cb in range(NCB):
                nc.tensor.matmul(
                    psk[:],
                    lhsT=kwT_sb[:, cb, :],
                    rhs=xts[cb][:],
                    start=(cb == 0),
                    stop=(cb == NCB - 1),
                )
            nc.scalar.copy(KT_sb[:, t0 : t0 + CH], psk[:])

        def project_v(j, xts):
            t0 = j * CH
            for tb in range(CH // P):
                gtb = j * (CH // P) + tb
                psv = ps_pool.tile([P, P], F32, name=f"psv_{j}_{tb}", tag="ps")
                for cb in range(NCB):
                    nc.tensor.matmul(
                        psv[:],
                        lhsT=xts[cb][:, tb * P : (tb + 1) * P],
                        rhs=vwT_sb[:, cb, :],
                        start=(cb == 0),
                        stop=(cb == NCB - 1),
                    )
                nc.vector.tensor_copy(Vaug_sb[:, gtb, 0:128], psv[:])

        def oproj_tblock(tb):
            for nch in range(C // CH):
                pso = ps_pool.tile([P, CH], F32, name=f"pso_{tb}_{nch}", tag="ps")
                for hb in range(NQH):
                    nc.tensor.matmul(
                        pso[:],
                        lhsT=yT_sb[:, hb, tb * P : (tb + 1) * P],
                        rhs=owT_sb[:, hb, nch * CH : (nch + 1) * CH],
                        start=(hb == 0),
                        stop=(hb == NQH - 1),
                    )
                ot = oev_pool.tile([P, CH], F16, name=f"ot_{tb}_{nch}", tag="ot")
                if (tb + nch) % 2 == 0:
                    nc.scalar.copy(ot[:], pso[:])
                else:
                    nc.vector.tensor_copy(ot[:], pso[:])
                nc.sync.dma_start(
                    out_ap[tb * P : (tb + 1) * P, nch * CH : (nch + 1) * CH], ot[:]
                )

        def attention_chunk(j, pending_tbs=(), fillers=()):
            q0 = j * CH
            nkb = 4 * j + 4
            pending_tbs = list(pending_tbs)
            fillers = list(fillers)
            # ALiBi band limit for local head slot 0 (the narrowest-slope head
            # of every group is ordered first: h0/h4/h8/h12, slopes >= 0.011).
            # Blocks with q-k >= 11*128 contribute < exp(-14) of softmax mass;
            # skip their scores/AV entirely.
            kb_lo_of = lambda h: max(0, 4 * j - 7) if h == 0 else 0

            def emit_scores(h):
                """Emit score matmul + mask-add + exp for one kb block; a
                generator so AV work of the previous head can be staggered
                between score blocks (keeps the in-order PE queue from
                stalling on the DVE->ACT softmax chain)."""
                ets = {}
                for i, kb in enumerate(range(kb_lo_of(h), nkb)):
                    oi = kb - 4 * j
                    # q-columns below oi*P are fully causal-masked; skip them
                    qoff = oi * P if oi > 0 else 0
                    pss = ps_pool.tile([P, CH], F32, name=f"pss_{h}_{j}_{kb}", tag="ps")
                    nc.tensor.matmul(
                        pss[:, qoff:],
                        lhsT=KT_sb[:, kb * P : (kb + 1) * P],
                        rhs=QT_sb[:, h, q0 + qoff : q0 + CH],
                        start=True,
                        stop=True,
                    )
                    t1 = t1_pool.tile([P, CH], F16, name=f"t1_{h}_{j}_{kb}", tag="t1")
                    oidx = oi + 12
                    # t1 = pss*(SCALE/slope) + (k-q); exp(slope*t1) below.
                    # (offloading adds to GPSIMD fails in walrus codegen:
                    # GPSIMD cannot read PSUM operands here)
                    nc.vector.scalar_tensor_tensor(
                        t1[:, qoff:],
                        pss[:, qoff:],
                        sc_sb[:, h, 0:1],
                        m_sb[:, oidx, qoff:],
                        op0=mybir.AluOpType.mult,
                        op1=mybir.AluOpType.add,
                    )
                    et = exp_pool.tile([P, CH], F16, name=f"et_{h}_{j}_{kb}", tag="et")
                    nc.scalar.activation(
                        et[:, qoff:],
                        t1[:, qoff:],
                        EXP,
                        scale=sc_sb[:, h, 1:2],
                    )
                    ets[kb] = et
                    yield ets

            def make_av_emitter(h):
                """Per-qb AV chain + delayed-normalize + transpose, with the
                PE transpose of block qb staggered behind AV of block qb+1 so
                it never heads the PE queue while its DVE inputs are pending."""
                pend = []

                def flush(n):
                    while len(pend) > n:
                        gqb, yps = pend.pop(0)
                        recip = rc_pool.tile([P, 1], F32, name=f"rc_{h}_{gqb}", tag="rc")
                        nc.vector.reciprocal(recip[:], yps[:, 128:129])
                        yn = yn_pool.tile([P, P], F16, name=f"yn_{h}_{gqb}", tag="yn")
                        nc.vector.tensor_scalar_mul(yn[:], yps[:, 0:128], recip[:])
                        tp = tp_pool.tile([P, P], F16, name=f"tp_{h}_{gqb}", tag="tp")
                        nc.tensor.transpose(tp[:], yn[:], id_sb[:])
                        nc.vector.tensor_copy(
                            yT_sb[:, h, gqb * P : (gqb + 1) * P], tp[:]
                        )

                def emit_one(qb, ets):
                    gqb = j * (CH // P) + qb
                    lo = kb_lo_of(h)
                    yps = yps_pool.tile([P, 132], F32, name=f"yps_{h}_{gqb}", tag="yps")
                    for kb in range(lo, gqb + 1):
                        nc.tensor.matmul(
                            yps[:, 0:129],
                            lhsT=ets[kb][:, qb * P : (qb + 1) * P],
                            rhs=Vaug_sb[:, kb, 0:129],
                            start=(kb == lo),
                            stop=(kb == gqb),
                        )
                    pend.append((gqb, yps))
                    flush(1)

                return emit_one, flush

            # NOTE: interleaving AV chains into the score loop measures ~5us
            # WORSE: a short 129-stream before a fat score matmul exposes the
            # fat matmul's weight load (LDW conservation) — keep smalls grouped.
            # DVE (mask-adds) and ACT (exps) saturate inside attention spans
            # while the PE has spare capacity, so previous-chunk o-projection
            # t-blocks are spread one-per-head here: independent fat PE work
            # that keeps the PE busy while the softmax chain drains.
            # Cross-head software pipeline: while head h's AV chain runs
            # (short dependent matmuls), head h+1's fat score matmuls are
            # interleaved so the in-order PE queue always has independent
            # work while the DVE->ACT softmax chain of h+1 drains.
            gens = [emit_scores(h) for h in range(NQH)]
            etss = [None] * NQH
            for etss[0] in gens[0]:
                pass
            nsc = nkb  # score blocks to advance per head (>= emitted count)
            for h in range(NQH):
                emit_one, flush = make_av_emitter(h)
                nxt = gens[h + 1] if h + 1 < NQH else None
                step = (nsc + CH // P - 1) // (CH // P)
                last = (j == NCHK - 1) and (h == NQH - 1)
                for qb in range(CH // P):
                    emit_one(qb, etss[h])
                    if nxt is not None:
                        for _ in range(step):
                            try:
                                etss[h + 1] = next(nxt)
                            except StopIteration:
                                break
                    if last:
                        if qb == 0:
                            if pending_tbs:
                                oproj_tblock(pending_tbs.pop(0))
                        else:
                            oproj_tblock(j * (CH // P) + qb - 1)
                if nxt is not None:
                    for etss[h + 1] in nxt:
                        pass
                if not last:
                    if pending_tbs:
                        oproj_tblock(pending_tbs.pop(0))
                    if fillers:
                        fillers.pop(0)()
                flush(0)
                if last:
                    oproj_tblock(j * (CH // P) + (CH // P) - 1)
            for tb in pending_tbs:
                oproj_tblock(tb)
            for f in fillers:
                f()

        # ---- prologue: chunk-0 projections, DMA-order-tuned ----
        xts_cur = project_x(0)
        nc.sync.dma_start(kwT_sb[:], kwT_ap[:])
        nc.sync.dma_start(sc_sb[:], sc_ap[:])
        # chunk 0's blocks are all diagonal (oi >= 0): that mask slice plus
        # sc is only 0.5MB -- land it before the fat weight tail so
        # attention(0) never waits ~20us on the full mask tensor.
        nc.sync.dma_start(m_sb[:, 12:16, :], m_ap[:, 12:16, :])
        nc.sync.dma_start(vwT_sb[:], vwT_ap[:])
        nc.sync.dma_start(qwT_sb[:, :, P:], qwT_ap[:, :, P:])
        qproj_unit(0, 0, xts_cur)
        project_k(0, xts_cur)
        project_v(0, xts_cur)
        for qh in range(1, NQH):
            qproj_unit(0, qh, xts_cur)
        # mask/bias tiles next: attention chunk 0 stalls on them well before
        # the x prefetch of chunk 1 or owT are needed.
        nc.sync.dma_start(m_sb[:, 0:12, :], m_ap[:, 0:12, :])

        for j in range(NCHK):
            if j + 1 < NCHK:
                xts_next = project_x(j + 1)
                # Q units of chunk j+1 + its K unit fill attention(j) head
                # boundaries (K early so the KT evac latency hides before
                # scores(j+1)).  Chunk 0 gets no fillers: its x prefetch is
                # still in flight and a stalling filler heads the in-order
                # PE queue (measured 1.5us gaps).
                units = [
                    (lambda jj=j + 1, qq=qh, xx=xts_next: qproj_unit(jj, qq, xx))
                    for qh in range(3)
                ] + [lambda jj=j + 1, xx=xts_next: project_k(jj, xx)]
                fillers = units if j > 0 else []
                post = [] if j > 0 else units
            else:
                xts_next = None
                fillers, post = [], []
            if j == 0:
                nc.sync.dma_start(owT_sb[:], owT_ap[:])
            # o-projection t-blocks of chunk j-1 are spread inside
            # attention(j), one per head (see attention_chunk).
            prev_tbs = range((j - 1) * (CH // P), j * (CH // P)) if j > 0 else ()
            attention_chunk(j, prev_tbs, fillers=fillers)
            for f in post:
                f()
            if xts_next is not None:
                qproj_unit(j + 1, 3, xts_next)
                project_v(j + 1, xts_next)

    nc.compile()
    return nc


def make_in_maps(x, q_w, k_w, v_w, o_w):
    """Host-side sharding/preprocessing -> per-core input dicts."""
    slopes = _alibi_slopes(H)
    x_bf = np.asarray(x, dtype=NP_F16)

    ident = np.eye(P, dtype=NP_F16)

    pi = np.arange(P, dtype=np.float32)[:, None]
    mj = np.arange(CH, dtype=np.float32)[None, :]

    # Relative-position mask, shared by all heads/cores: exact small ints
    # in f16 (|k-q| <= 2047 < 2048), causal entries -> -60000 (underflows
    # the f16 exp for every slope).
    mrel = np.empty((P, 16, CH), dtype=np.float32)
    for oidx in range(16):
        X = (oidx - 12) * P + pi - mj
        mrel[:, oidx, :] = np.where(X > 0.0, np.float32(MASK_NEG), X)
    mrel = mrel.astype(np.float16)

    in_maps = []
    for c in range(8):
        b, g = c // 4, c % 4
        qsl = slice(4 * g * P, (4 * g + 4) * P)
        ksl = slice(g * P, (g + 1) * P)

        qwT = np.ascontiguousarray(
            np.asarray(q_w[qsl].T, dtype=NP_F16).reshape(NCB, P, NQH * P).transpose(1, 0, 2)
        )
        kwT = np.ascontiguousarray(
            np.asarray(k_w[ksl].T, dtype=NP_F16).reshape(NCB, P, D).transpose(1, 0, 2)
        )
        vwT = np.ascontiguousarray(
            np.asarray(v_w[ksl].T, dtype=NP_F16).reshape(NCB, P, D).transpose(1, 0, 2)
        )
        owT = np.ascontiguousarray(
            np.asarray(o_w[:, qsl].T, dtype=NP_F16).reshape(NQH, P, C).transpose(1, 0, 2)
        )

        sctbl = np.empty((P, NQH, 2), dtype=np.float32)
        for h in range(NQH):
            sl = np.float64(slopes[4 * g + h])
            sctbl[:, h, 0] = np.float32(SCALE / sl)
            sctbl[:, h, 1] = np.float32(sl)

        in_maps.append(
            dict(
                xT=np.ascontiguousarray(x_bf[b].T),
                qwT=qwT,
                kwT=kwT,
                vwT=vwT,
                owT=owT,
                mrel=mrel,
                sctbl=sctbl,
                ident=ident,
            )
        )
    return in_maps


def gather_output(results):
    out = np.zeros((B, T, C), dtype=np.float32)
    for c in range(8):
        out[c // 4] += results[c]["out_p"].astype(np.float32)
    return out


_NC_CACHE = {}


def get_program():
    if "nc" not in _NC_CACHE:
        _NC_CACHE["nc"] = build_program()
    return _NC_CACHE["nc"]


def kernel(x, q_w, k_w, v_w, o_w):
    from concourse.bass_utils import run_bass_kernel_spmd

    nc = get_program()
    in_maps = make_in_maps(x, q_w, k_w, v_w, o_w)
    res = run_bass_kernel_spmd(nc, in_maps, list(range(8)))
    return gather_output(res.results)



# revision 24
# speedup vs baseline: 1.1874x; 1.1874x over previous
"""Trainium2 Bass kernel for causal self-attention with ALiBi + GQA.

Problem: B=2, T=2048, C=2048, 16 q-heads / 4 kv-heads, head_dim=128.
  q = x@q_w.T, k = x@k_w.T, v = x@v_w.T (GQA repeat 4x)
  att = softmax(q k^T/sqrt(d) + causal + alibi); out = (att v) @ o_w.T

Sharding over 8 NeuronCores: core c -> batch c//4, kv-group g=c%4
(q-heads 4g..4g+3, kv-head g).  Each core computes attention for its 4
heads on its batch plus a partial o-projection over its 512 channels;
the host sums the 4 partials per batch.

On-chip design (per core, all matmuls fp16, fp32 accumulate — fp16 is
1 cycle/row on the PE like bf16 but with 4x the mantissa; all values
here are bounded so there is no range risk):
  - x is host-cast to fp16 and host-TRANSPOSED (xT [C,T]) so projection
    moving operands load with plain contiguous DMA.
  - Projections make QT [d,t], KT [d,t] (transposed) and V natural
    [t,d] with a ones-column appended, so the AV matmul also emits the
    softmax denominator for free.
  - Scores are computed transposed sT[k,q] = KTblk.T @ QT (moving free
    dim 512, causally narrowed per diagonal offset); ALiBi + causal
    folded in via host-precomputed additive f32 tiles (DVE) and a
    per-(head,offset) bias in the ACT exp.  No max-subtraction needed:
    scores are small (~N(0,0.8)) and masked entries use -1e9.
  - ALiBi+causal masking uses ONE head-independent relative-position
    table M[pi,oidx,mj] = k-q (exact small ints in f16; causal ->
    -60000) plus per-core [P,1] scalar tables: DVE fused
    scalar_tensor_tensor computes t1 = pss*(SCALE/slope) + (k-q) in
    f16, ACT exp applies scale=slope from an AP.  Storing k-q directly
    (instead of slope-scaled masks) keeps f16 rounding error ~5e-4 of
    the EXPONENT at the entries that matter -- slope-scaled f16 masks
    lose 1e-2 to large-term cancellation.  Far blocks underflow exp to
    0, which is exactly the negligible tail of the softmax.
  - ALiBi band sparsity: the narrowest-band head of each group is
    ordered first (h0/h4/h8/h12, slopes >= 0.011); its score/AV blocks
    with q-k >= 11*128 (softmax mass < e^-14) are skipped.
  - y accumulates un-normalized; delayed normalization via per-row
    reciprocal of the ones-column sums, then PE-transpose -> yT feeds
    the o-projection (psum -> ACT/DVE copy -> fp16 -> DMA out; host
    sums the 4 partials per batch in fp32).
  - Scheduling: a warmup matmul burst ramps the PE p-state; chunk
    j+1's x tiles are prefetched and its per-head Q-projection units
    emitted at attention(j) head boundaries (with prev-chunk
    o-projection t-blocks) BEFORE the yT flush, so the in-order PE
    queue always has fat independent work while the DVE->ACT softmax
    chain drains; head h+1's score matmuls are software-pipelined
    into head h's AV chain; the last chunk's o-projection t-blocks
    interleave into its last head's AV loop (no serial tail); chunk
    0's diagonal mask slice (0.5MB) is DMA'd before the fat weight
    tail so attention(0) never waits on the full 2MB mask tensor.
Measured: ~266-268 us/core on TRN2 (NTFF; occasional +15-50us
outlier runs from device contention), L2 rel err 6.6e-4 vs fp32
reference.  (Baseline of this session: 272-274us, 6.3e-4.)

Rejected directions (measured on this HW): AV with V-stationary (fat
512-wide moving) needs softmax denominators off-PE — GPSIMD add-chains
+ partition_all_reduce are ~4x slower than the cost model (634us total);
XBAR dma_start_transpose from SBUF (direct or DRAM-bounced) returns
wrong data (NaN) in this axon environment; matmul moving free dim is
hard-capped at 512 so wider PSUM tiles are impossible. The ones-column
AV is LDW-conservation-optimal: any orientation computing denominators
on-PE needs the same 544 stationary loads.
"""

import math
import sys
from contextlib import ExitStack

import numpy as np

sys.path.insert(0, "/opt/trn_rl_repo")

import ml_dtypes  # noqa: E402

import concourse.bacc as bacc  # noqa: E402
import concourse.bass as bass  # noqa: E402
import concourse.mybir as mybir  # noqa: E402
import concourse.tile as tile  # noqa: E402

F16 = mybir.dt.float16
F32 = mybir.dt.float32
NP_F16 = np.float16

B, T, C = 2, 2048, 2048
H, HKV, D = 16, 4, 128
P = 128
CH = 512                 # q-chunk (moving free dim)
NCB = C // P             # 16 contraction blocks
NTB = T // P             # 16 t-blocks
NCHK = T // CH           # 4 q-chunks
NQH = 4                  # local q heads per core
SCALE = 1.0 / math.sqrt(D)
MASK_NEG = -60000.0     # f16-representable; exp(scale*(-60000)) == 0


def _alibi_slopes(n):
    start = 2 ** (-(2 ** (-(math.log2(n) - 3))))
    return np.array([start * start**i for i in range(n)], dtype=np.float64)


def build_program():
    """Build the (SPMD-identical) single-core program."""
    nc = bacc.Bacc("TRN2", target_bir_lowering=False, debug=False, num_devices=8)

    xT_ap = nc.dram_tensor("xT", [C, T], F16, kind="ExternalInput").ap()
    qwT_ap = nc.dram_tensor("qwT", [P, NCB, NQH * P], F16, kind="ExternalInput").ap()
    kwT_ap = nc.dram_tensor("kwT", [P, NCB, D], F16, kind="ExternalInput").ap()
    vwT_ap = nc.dram_tensor("vwT", [P, NCB, D], F16, kind="ExternalInput").ap()
    owT_ap = nc.dram_tensor("owT", [P, NQH, C], F16, kind="ExternalInput").ap()
    # M[pi, oidx, mj] = k - q (exact small ints in f16) with causal -60000;
    # head-independent: the slope is applied via per-core scalar tables.
    m_ap = nc.dram_tensor("mrel", [P, 16, CH], F16, kind="ExternalInput").ap()
    sc_ap = nc.dram_tensor("sctbl", [P, NQH, 2], F32, kind="ExternalInput").ap()
    id_ap = nc.dram_tensor("ident", [P, P], F16, kind="ExternalInput").ap()
    out_ap = nc.dram_tensor("out_p", [T, C], F16, kind="ExternalOutput").ap()

    EXP = mybir.ActivationFunctionType.Exp

    with tile.TileContext(nc) as tc, ExitStack() as ctx:
        const = ctx.enter_context(tc.tile_pool(name="const", bufs=1))
        qwT_sb = const.tile([P, NCB, NQH * P], F16, name="qwT_sb")
        kwT_sb = const.tile([P, NCB, D], F16, name="kwT_sb")
        vwT_sb = const.tile([P, NCB, D], F16, name="vwT_sb")
        owT_sb = const.tile([P, NQH, C], F16, name="owT_sb")
        m_sb = const.tile([P, 16, CH], F16, name="m_sb")
        sc_sb = const.tile([P, NQH, 2], F32, name="sc_sb")
        id_sb = const.tile([P, P], F16, name="id_sb")

        wid_sb = const.tile([P, P], F16, name="wid_sb")
        QT_sb = const.tile([P, NQH, T], F16, name="QT_sb")
        KT_sb = const.tile([P, T], F16, name="KT_sb")
        Vaug_sb = const.tile([P, NTB, 132], F16, name="Vaug_sb")
        yT_sb = const.tile([P, NQH, T], F16, name="yT_sb")

        # DMA order tuned so warmup + the first productive matmul (Q-proj
        # head 0 of chunk 0) start as early as possible: tiny identity
        # first (warmup reads it), qwT head-0 slice next, then the chunk-0
        # x tiles (issued by project_x below), then the rest.
        nc.sync.dma_start(id_sb[:], id_ap[:])
        nc.sync.dma_start(qwT_sb[:, :, 0:P], qwT_ap[:, :, 0:P])

        nc.vector.memset(Vaug_sb[:, :, 128:129], 1.0)

        xT_pool = ctx.enter_context(tc.tile_pool(name="xT_pool", bufs=32))
        ps_pool = ctx.enter_context(tc.tile_pool(name="ps_pool", bufs=4, space="PSUM"))
        yps_pool = ctx.enter_context(tc.tile_pool(name="yps_pool", bufs=3, space="PSUM"))
        tp_pool = ctx.enter_context(tc.tile_pool(name="tp_pool", bufs=1, space="PSUM"))
        t1_pool = ctx.enter_context(tc.tile_pool(name="t1_pool", bufs=8))
        exp_pool = ctx.enter_context(tc.tile_pool(name="exp_pool", bufs=34))
        oev_pool = ctx.enter_context(tc.tile_pool(name="oev_pool", bufs=8))
        yn_pool = ctx.enter_context(tc.tile_pool(name="yn_pool", bufs=6))
        rc_pool = ctx.enter_context(tc.tile_pool(name="rc_pool", bufs=6))

        # PE p-state warm-up: dependency-free dummy matmuls ramp the tensor
        # engine to full clock while the first input DMAs land.
        nc.vector.memset(wid_sb[:], 0.0)
        for w in range(40):
            wups = yps_pool.tile([P, 132], F32, name=f"wups_{w}", tag="yps")
            nc.tensor.matmul(
                wups[:, 0:128], lhsT=wid_sb[:], rhs=wid_sb[:], start=True, stop=True
            )

        # ---- Fused per-chunk pipeline: project chunk j, then attention for
        # chunk j (legal because causality means queries in chunk j only
        # attend to keys/values t <= chunk j), then its o-projection.
        # Chunk j+1's x tiles are prefetched and its per-head Q-projection
        # units emitted INSIDE attention(j) at head boundaries: independent
        # fat PE work that absorbs the DVE/ACT softmax-chain latency. ----
        def project_x(j):
            t0 = j * CH
            xts = []
            for cb in range(NCB):
                xt = xT_pool.tile([P, CH], F16, name=f"xt_{j}_{cb}", tag="xt")
                nc.sync.dma_start(xt[:], xT_ap[cb * P : (cb + 1) * P, t0 : t0 + CH])
                xts.append(xt)
            return xts

        def qproj_unit(j, qh, xts):
            t0 = j * CH
            ps = ps_pool.tile([P, CH], F32, name=f"psq_{j}_{qh}", tag="ps")
            for cb in range(NCB):
                nc.tensor.matmul(
                    ps[:],
                    lhsT=qwT_sb[:, cb, qh * P : (qh + 1) * P],
                    rhs=xts[cb][:],
                    start=(cb == 0),
                    stop=(cb == NCB - 1),
                )
            # Scalar-engine evac: frees DVE (mask-adds) and unblocks the
            # PSUM pool faster so downstream matmuls don't stall on buffers.
            nc.scalar.copy(QT_sb[:, qh, t0 : t0 + CH], ps[:])

        def project_k(j, xts):
            t0 = j * CH
            psk = ps_pool.tile([P, CH], F32, name=f"psk_{j}", tag="ps")
            for cb in range(NCB):
                nc.tensor.matmul(
                    psk[:],
                    lhsT=kwT_sb[:, cb, :],
                    rhs=xts[cb][:],
                    start=(cb == 0),
                    stop=(cb == NCB - 1),
                )
            nc.scalar.copy(KT_sb[:, t0 : t0 + CH], psk[:])

        def project_v(j, xts):
            t0 = j * CH
            for tb in range(CH // P):
                gtb = j * (CH // P) + tb
                psv = ps_pool.tile([P, P], F32, name=f"psv_{j}_{tb}", tag="ps")
                for cb in range(NCB):
                    nc.tensor.matmul(
                        psv[:],
                        lhsT=xts[cb][:, tb * P : (tb + 1) * P],
                        rhs=vwT_sb[:, cb, :],
                        start=(cb == 0),
                        stop=(cb == NCB - 1),
                    )
                nc.vector.tensor_copy(Vaug_sb[:, gtb, 0:128], psv[:])

        def oproj_tblock(tb):
            for nch in range(C // CH):
                pso = ps_pool.tile([P, CH], F32, name=f"pso_{tb}_{nch}", tag="ps")
                for hb in range(NQH):
                    nc.tensor.matmul(
                        pso[:],
                        lhsT=yT_sb[:, hb, tb * P : (tb + 1) * P],
                        rhs=owT_sb[:, hb, nch * CH : (nch + 1) * CH],
                        start=(hb == 0),
                        stop=(hb == NQH - 1),
                    )
                ot = oev_pool.tile([P, CH], F16, name=f"ot_{tb}_{nch}", tag="ot")
                if (tb + nch) % 2 == 0:
                    nc.scalar.copy(ot[:], pso[:])
                else:
                    nc.vector.tensor_copy(ot[:], pso[:])
                nc.sync.dma_start(
                    out_ap[tb * P : (tb + 1) * P, nch * CH : (nch + 1) * CH], ot[:]
                )

        def attention_chunk(j, pending_tbs=(), fillers=()):
            q0 = j * CH
            nkb = 4 * j + 4
            pending_tbs = list(pending_tbs)
            fillers = list(fillers)
            # ALiBi band limit for local head slot 0 (the narrowest-slope head
            # of every group is ordered first: h0/h4/h8/h12, slopes >= 0.011).
            # Blocks with q-k >= 11*128 contribute < exp(-14) of softmax mass;
            # skip their scores/AV entirely.
            kb_lo_of = lambda h: max(0, 4 * j - 7) if h == 0 else 0

            def emit_scores(h):
                """Emit score matmul + mask-add + exp for one kb block; a
                generator so AV work of the previous head can be staggered
                between score blocks (keeps the in-order PE queue from
                stalling on the DVE->ACT softmax chain)."""
                ets = {}
                for i, kb in enumerate(range(kb_lo_of(h), nkb)):
                    oi = kb - 4 * j
                    # q-columns below oi*P are fully causal-masked; skip them
                    qoff = oi * P if oi > 0 else 0
                    pss = ps_pool.tile([P, CH], F32, name=f"pss_{h}_{j}_{kb}", tag="ps")
                    nc.tensor.matmul(
                        pss[:, qoff:],
                        lhsT=KT_sb[:, kb * P : (kb + 1) * P],
                        rhs=QT_sb[:, h, q0 + qoff : q0 + CH],
                        start=True,
                        stop=True,
                    )
                    t1 = t1_pool.tile([P, CH], F16, name=f"t1_{h}_{j}_{kb}", tag="t1")
                    oidx = oi + 12
                    # t1 = pss*(SCALE/slope) + (k-q); exp(slope*t1) below.
                    # (offloading adds to GPSIMD fails in walrus codegen:
                    # GPSIMD cannot read PSUM operands here)
                    nc.vector.scalar_tensor_tensor(
                        t1[:, qoff:],
                        pss[:, qoff:],
                        sc_sb[:, h, 0:1],
                        m_sb[:, oidx, qoff:],
                        op0=mybir.AluOpType.mult,
                        op1=mybir.AluOpType.add,
                    )
                    et = exp_pool.tile([P, CH], F16, name=f"et_{h}_{j}_{kb}", tag="et")
                    nc.scalar.activation(
                        et[:, qoff:],
                        t1[:, qoff:],
                        EXP,
                        scale=sc_sb[:, h, 1:2],
                    )
                    ets[kb] = et
                    yield ets

            def make_av_emitter(h):
                """Per-qb AV chain + delayed-normalize + transpose, with the
                PE transpose of block qb staggered behind AV of block qb+1 so
                it never heads the PE queue while its DVE inputs are pending."""
                pend = []

                def flush(n):
                    while len(pend) > n:
                        gqb, yps = pend.pop(0)
                        recip = rc_pool.tile([P, 1], F32, name=f"rc_{h}_{gqb}", tag="rc")
                        nc.vector.reciprocal(recip[:], yps[:, 128:129])
                        yn = yn_pool.tile([P, P], F16, name=f"yn_{h}_{gqb}", tag="yn")
                        nc.vector.tensor_scalar_mul(yn[:], yps[:, 0:128], recip[:])
                        tp = tp_pool.tile([P, P], F16, name=f"tp_{h}_{gqb}", tag="tp")
                        nc.tensor.transpose(tp[:], yn[:], id_sb[:])
                        nc.vector.tensor_copy(
                            yT_sb[:, h, gqb * P : (gqb + 1) * P], tp[:]
                        )

                def emit_one(qb, ets):
                    gqb = j * (CH // P) + qb
                    lo = kb_lo_of(h)
                    yps = yps_pool.tile([P, 132], F32, name=f"yps_{h}_{gqb}", tag="yps")
                    for kb in range(lo, gqb + 1):
                        nc.tensor.matmul(
                            yps[:, 0:129],
                            lhsT=ets[kb][:, qb * P : (qb + 1) * P],
                            rhs=Vaug_sb[:, kb, 0:129],
                            start=(kb == lo),
                            stop=(kb == gqb),
                        )
                    pend.append((gqb, yps))
                    flush(1)

                return emit_one, flush

            # NOTE: interleaving AV chains into the score loop measures ~5us
            # WORSE: a short 129-stream before a fat score matmul exposes the
            # fat matmul's weight load (LDW conservation) — keep smalls grouped.
            # DVE (mask-adds) and ACT (exps) saturate inside attention spans
            # while the PE has spare capacity, so previous-chunk o-projection
            # t-blocks are spread one-per-head here: independent fat PE work
            # that keeps the PE busy while the softmax chain drains.
            # Cross-head software pipeline: while head h's AV chain runs
            # (short dependent matmuls), head h+1's fat score matmuls are
            # interleaved so the in-order PE queue always has independent
            # work while the DVE->ACT softmax chain of h+1 drains.
            gens = [emit_scores(h) for h in range(NQH)]
            etss = [None] * NQH
            for etss[0] in gens[0]:
                pass
            nsc = nkb  # score blocks to advance per head (>= emitted count)
            for h in range(NQH):
                emit_one, flush = make_av_emitter(h)
                nxt = gens[h + 1] if h + 1 < NQH else None
                step = (nsc + CH // P - 1) // (CH // P)
                last = (j == NCHK - 1) and (h == NQH - 1)
                for qb in range(CH // P):
                    emit_one(qb, etss[h])
                    if nxt is not None:
                        for _ in range(step):
                            try:
                                etss[h + 1] = next(nxt)
                            except StopIteration:
                                break
                    if last:
                        if qb == 0:
                            if pending_tbs:
                                oproj_tblock(pending_tbs.pop(0))
                        else:
                            oproj_tblock(j * (CH // P) + qb - 1)
                if nxt is not None:
                    for etss[h + 1] in nxt:
                        pass
                if not last:
                    if pending_tbs:
                        oproj_tblock(pending_tbs.pop(0))
                    if fillers:
                        fillers.pop(0)()
                flush(0)
                if last:
                    oproj_tblock(j * (CH // P) + (CH // P) - 1)
            for tb in pending_tbs:
                oproj_tblock(tb)
            for f in fillers:
                f()

        # ---- prologue: chunk-0 projections, DMA-order-tuned ----
        xts_cur = project_x(0)
        nc.sync.dma_start(kwT_sb[:], kwT_ap[:])
        nc.sync.dma_start(sc_sb[:], sc_ap[:])
        # chunk 0's blocks are all diagonal (oi >= 0): that mask slice plus
        # sc is only 0.5MB -- land it before the fat weight tail so
        # attention(0) never waits ~20us on the full mask tensor.
        nc.sync.dma_start(m_sb[:, 12:16, :], m_ap[:, 12:16, :])
        nc.sync.dma_start(vwT_sb[:], vwT_ap[:])
        nc.sync.dma_start(qwT_sb[:, :, P:], qwT_ap[:, :, P:])
        qproj_unit(0, 0, xts_cur)
        project_k(0, xts_cur)
        project_v(0, xts_cur)
        for qh in range(1, NQH):
            qproj_unit(0, qh, xts_cur)
        # mask/bias tiles next: attention chunk 0 stalls on them well before
        # the x prefetch of chunk 1 or owT are needed.
        nc.sync.dma_start(m_sb[:, 0:12, :], m_ap[:, 0:12, :])

        for j in range(NCHK):
            if j + 1 < NCHK:
                xts_next = project_x(j + 1)
                fillers = [
                    (lambda jj=j + 1, qq=qh, xx=xts_next: qproj_unit(jj, qq, xx))
                    for qh in range(NQH)
                ]
            else:
                xts_next = None
                fillers = []
            if j == 0:
                nc.sync.dma_start(owT_sb[:], owT_ap[:])
            # o-projection t-blocks of chunk j-1 are spread inside
            # attention(j), one per head (see attention_chunk).
            prev_tbs = range((j - 1) * (CH // P), j * (CH // P)) if j > 0 else ()
            attention_chunk(j, prev_tbs, fillers=fillers)
            if xts_next is not None:
                project_k(j + 1, xts_next)
                project_v(j + 1, xts_next)

    nc.compile()
    return nc


def make_in_maps(x, q_w, k_w, v_w, o_w):
    """Host-side sharding/preprocessing -> per-core input dicts."""
    slopes = _alibi_slopes(H)
    x_bf = np.asarray(x, dtype=NP_F16)

    ident = np.eye(P, dtype=NP_F16)

    pi = np.arange(P, dtype=np.float32)[:, None]
    mj = np.arange(CH, dtype=np.float32)[None, :]

    # Relative-position mask, shared by all heads/cores: exact small ints
    # in f16 (|k-q| <= 2047 < 2048), causal entries -> -60000 (underflows
    # the f16 exp for every slope).
    mrel = np.empty((P, 16, CH), dtype=np.float32)
    for oidx in range(16):
        X = (oidx - 12) * P + pi - mj
        mrel[:, oidx, :] = np.where(X > 0.0, np.float32(MASK_NEG), X)
    mrel = mrel.astype(np.float16)

    in_maps = []
    for c in range(8):
        b, g = c // 4, c % 4
        qsl = slice(4 * g * P, (4 * g + 4) * P)
        ksl = slice(g * P, (g + 1) * P)

        qwT = np.ascontiguousarray(
            np.asarray(q_w[qsl].T, dtype=NP_F16).reshape(NCB, P, NQH * P).transpose(1, 0, 2)
        )
        kwT = np.ascontiguousarray(
            np.asarray(k_w[ksl].T, dtype=NP_F16).reshape(NCB, P, D).transpose(1, 0, 2)
        )
        vwT = np.ascontiguousarray(
            np.asarray(v_w[ksl].T, dtype=NP_F16).reshape(NCB, P, D).transpose(1, 0, 2)
        )
        owT = np.ascontiguousarray(
            np.asarray(o_w[:, qsl].T, dtype=NP_F16).reshape(NQH, P, C).transpose(1, 0, 2)
        )

        sctbl = np.empty((P, NQH, 2), dtype=np.float32)
        for h in range(NQH):
            sl = np.float64(slopes[4 * g + h])
            sctbl[:, h, 0] = np.float32(SCALE / sl)
            sctbl[:, h, 1] = np.float32(sl)

        in_maps.append(
            dict(
                xT=np.ascontiguousarray(x_bf[b].T),
                qwT=qwT,
                kwT=kwT,
                vwT=vwT,
                owT=owT,
                mrel=mrel,
                sctbl=sctbl,
                ident=ident,
            )
        )
    return in_maps


def gather_output(results):
    out = np.zeros((B, T, C), dtype=np.float32)
    for c in range(8):
        out[c // 4] += results[c]["out_p"].astype(np.float32)
    return out


_NC_CACHE = {}


def get_program():
    if "nc" not in _NC_CACHE:
        _NC_CACHE["nc"] = build_program()
    return _NC_CACHE["nc"]


def kernel(x, q_w, k_w, v_w, o_w):
    from concourse.bass_utils import run_bass_kernel_spmd

    nc = get_program()
    in_maps = make_in_maps(x, q_w, k_w, v_w, o_w)
    res = run_bass_kernel_spmd(nc, in_maps, list(range(8)))
    return gather_output(res.results)



# revision 25
# speedup vs baseline: 1.2093x; 1.0184x over previous
"""Trainium2 Bass kernel for causal self-attention with ALiBi + GQA.

Problem: B=2, T=2048, C=2048, 16 q-heads / 4 kv-heads, head_dim=128.
  q = x@q_w.T, k = x@k_w.T, v = x@v_w.T (GQA repeat 4x)
  att = softmax(q k^T/sqrt(d) + causal + alibi); out = (att v) @ o_w.T

Sharding over 8 NeuronCores: core c -> batch c//4, kv-group g=c%4
(q-heads 4g..4g+3, kv-head g).  Each core computes attention for its 4
heads on its batch plus a partial o-projection over its 512 channels;
the host sums the 4 partials per batch.

On-chip design (per core, all matmuls fp16, fp32 accumulate — fp16 is
1 cycle/row on the PE like bf16 but with 4x the mantissa; all values
here are bounded so there is no range risk):
  - x is host-cast to fp16 and host-TRANSPOSED (xT [C,T]) so projection
    moving operands load with plain contiguous DMA.
  - Projections make QT [d,t], KT [d,t] (transposed) and V natural
    [t,d] with a ones-column appended, so the AV matmul also emits the
    softmax denominator for free.
  - Scores are computed transposed sT[k,q] = KTblk.T @ QT (moving free
    dim 512, causally narrowed per diagonal offset); ALiBi + causal
    folded in via host-precomputed additive f32 tiles (DVE) and a
    per-(head,offset) bias in the ACT exp.  No max-subtraction needed:
    scores are small (~N(0,0.8)) and masked entries use -1e9.
  - ALiBi+causal masking uses ONE head-independent relative-position
    table M[pi,oidx,mj] = k-q (exact small ints in f16; causal ->
    -60000) plus per-core [P,1] scalar tables: DVE fused
    scalar_tensor_tensor computes t1 = pss*(SCALE/slope) + (k-q) in
    f16, ACT exp applies scale=slope from an AP.  Storing k-q directly
    (instead of slope-scaled masks) keeps f16 rounding error ~5e-4 of
    the EXPONENT at the entries that matter -- slope-scaled f16 masks
    lose 1e-2 to large-term cancellation.  Far blocks underflow exp to
    0, which is exactly the negligible tail of the softmax.
  - ALiBi band sparsity: the narrowest-band head of each group is
    ordered first (h0/h4/h8/h12, slopes >= 0.011); its score/AV blocks
    with q-k >= 11*128 (softmax mass < e^-14) are skipped.
  - y accumulates un-normalized; delayed normalization via per-row
    reciprocal of the ones-column sums, then PE-transpose -> yT feeds
    the o-projection (psum -> ACT/DVE copy -> fp16 -> DMA out; host
    sums the 4 partials per batch in fp32).
  - Scheduling: a warmup matmul burst ramps the PE p-state; chunk
    j+1's x tiles are prefetched and its per-head Q-projection units
    emitted at attention(j) head boundaries (with prev-chunk
    o-projection t-blocks) BEFORE the yT flush, so the in-order PE
    queue always has fat independent work while the DVE->ACT softmax
    chain drains; head h+1's score matmuls are software-pipelined
    into head h's AV chain; the last chunk's o-projection t-blocks
    interleave into its last head's AV loop (no serial tail); chunk
    0's diagonal mask slice (0.5MB) is DMA'd before the fat weight
    tail so attention(0) never waits on the full 2MB mask tensor.
Measured: ~266-268 us/core on TRN2 (NTFF; occasional +15-50us
outlier runs from device contention), L2 rel err 6.6e-4 vs fp32
reference.  (Baseline of this session: 272-274us, 6.3e-4.)

Rejected directions (measured on this HW): AV with V-stationary (fat
512-wide moving) needs softmax denominators off-PE — GPSIMD add-chains
+ partition_all_reduce are ~4x slower than the cost model (634us total);
XBAR dma_start_transpose from SBUF (direct or DRAM-bounced) returns
wrong data (NaN) in this axon environment; matmul moving free dim is
hard-capped at 512 so wider PSUM tiles are impossible. The ones-column
AV is LDW-conservation-optimal: any orientation computing denominators
on-PE needs the same 544 stationary loads.
"""

import math
import sys
from contextlib import ExitStack

import numpy as np

sys.path.insert(0, "/opt/trn_rl_repo")

import ml_dtypes  # noqa: E402

import concourse.bacc as bacc  # noqa: E402
import concourse.bass as bass  # noqa: E402
import concourse.mybir as mybir  # noqa: E402
import concourse.tile as tile  # noqa: E402

F16 = mybir.dt.float16
F32 = mybir.dt.float32
NP_F16 = np.float16

B, T, C = 2, 2048, 2048
H, HKV, D = 16, 4, 128
P = 128
CH = 512                 # q-chunk (moving free dim)
NCB = C // P             # 16 contraction blocks
NTB = T // P             # 16 t-blocks
NCHK = T // CH           # 4 q-chunks
NQH = 4                  # local q heads per core
SCALE = 1.0 / math.sqrt(D)
MASK_NEG = -60000.0     # f16-representable; exp(scale*(-60000)) == 0


def _alibi_slopes(n):
    start = 2 ** (-(2 ** (-(math.log2(n) - 3))))
    return np.array([start * start**i for i in range(n)], dtype=np.float64)


def build_program():
    """Build the (SPMD-identical) single-core program."""
    nc = bacc.Bacc("TRN2", target_bir_lowering=False, debug=False, num_devices=8)

    xT_ap = nc.dram_tensor("xT", [C, T], F16, kind="ExternalInput").ap()
    qwT_ap = nc.dram_tensor("qwT", [P, NCB, NQH * P], F16, kind="ExternalInput").ap()
    kwT_ap = nc.dram_tensor("kwT", [P, NCB, D], F16, kind="ExternalInput").ap()
    vwT_ap = nc.dram_tensor("vwT", [P, NCB, D], F16, kind="ExternalInput").ap()
    owT_ap = nc.dram_tensor("owT", [P, NQH, C], F16, kind="ExternalInput").ap()
    # M[pi, oidx, mj] = k - q (exact small ints in f16) with causal -60000;
    # head-independent: the slope is applied via per-core scalar tables.
    m_ap = nc.dram_tensor("mrel", [P, 16, CH], F16, kind="ExternalInput").ap()
    sc_ap = nc.dram_tensor("sctbl", [P, NQH, 2], F32, kind="ExternalInput").ap()
    id_ap = nc.dram_tensor("ident", [P, P], F16, kind="ExternalInput").ap()
    out_ap = nc.dram_tensor("out_p", [T, C], F16, kind="ExternalOutput").ap()

    EXP = mybir.ActivationFunctionType.Exp

    with tile.TileContext(nc) as tc, ExitStack() as ctx:
        const = ctx.enter_context(tc.tile_pool(name="const", bufs=1))
        qwT_sb = const.tile([P, NCB, NQH * P], F16, name="qwT_sb")
        kwT_sb = const.tile([P, NCB, D], F16, name="kwT_sb")
        vwT_sb = const.tile([P, NCB, D], F16, name="vwT_sb")
        owT_sb = const.tile([P, NQH, C], F16, name="owT_sb")
        m_sb = const.tile([P, 16, CH], F16, name="m_sb")
        sc_sb = const.tile([P, NQH, 2], F32, name="sc_sb")
        id_sb = const.tile([P, P], F16, name="id_sb")

        wid_sb = const.tile([P, P], F16, name="wid_sb")
        QT_sb = const.tile([P, NQH, T], F16, name="QT_sb")
        KT_sb = const.tile([P, T], F16, name="KT_sb")
        Vaug_sb = const.tile([P, NTB, 132], F16, name="Vaug_sb")
        yT_sb = const.tile([P, NQH, T], F16, name="yT_sb")

        # DMA order tuned so warmup + the first productive matmul (Q-proj
        # head 0 of chunk 0) start as early as possible: tiny identity
        # first (warmup reads it), qwT head-0 slice next, then the chunk-0
        # x tiles (issued by project_x below), then the rest.
        nc.sync.dma_start(id_sb[:], id_ap[:])
        nc.sync.dma_start(qwT_sb[:, :, 0:P], qwT_ap[:, :, 0:P])

        nc.vector.memset(Vaug_sb[:, :, 128:129], 1.0)

        xT_pool = ctx.enter_context(tc.tile_pool(name="xT_pool", bufs=32))
        ps_pool = ctx.enter_context(tc.tile_pool(name="ps_pool", bufs=5, space="PSUM"))
        yps_pool = ctx.enter_context(tc.tile_pool(name="yps_pool", bufs=2, space="PSUM"))
        tp_pool = ctx.enter_context(tc.tile_pool(name="tp_pool", bufs=1, space="PSUM"))
        t1_pool = ctx.enter_context(tc.tile_pool(name="t1_pool", bufs=8))
        exp_pool = ctx.enter_context(tc.tile_pool(name="exp_pool", bufs=34))
        oev_pool = ctx.enter_context(tc.tile_pool(name="oev_pool", bufs=8))
        yn_pool = ctx.enter_context(tc.tile_pool(name="yn_pool", bufs=6))
        rc_pool = ctx.enter_context(tc.tile_pool(name="rc_pool", bufs=6))

        # PE p-state warm-up: dependency-free dummy matmuls ramp the tensor
        # engine to full clock while the first input DMAs land.
        nc.vector.memset(wid_sb[:], 0.0)
        for w in range(100):
            wups = yps_pool.tile([P, 132], F32, name=f"wups_{w}", tag="yps")
            nc.tensor.matmul(
                wups[:, 0:128], lhsT=wid_sb[:], rhs=wid_sb[:], start=True, stop=True
            )

        # ---- Fused per-chunk pipeline: project chunk j, then attention for
        # chunk j (legal because causality means queries in chunk j only
        # attend to keys/values t <= chunk j), then its o-projection.
        # Chunk j+1's x tiles are prefetched and its per-head Q-projection
        # units emitted INSIDE attention(j) at head boundaries: independent
        # fat PE work that absorbs the DVE/ACT softmax-chain latency. ----
        def project_x(j):
            t0 = j * CH
            xts = []
            for cb in range(NCB):
                xt = xT_pool.tile([P, CH], F16, name=f"xt_{j}_{cb}", tag="xt")
                nc.sync.dma_start(xt[:], xT_ap[cb * P : (cb + 1) * P, t0 : t0 + CH])
                xts.append(xt)
            return xts

        def qproj_unit(j, qh, xts):
            t0 = j * CH
            ps = ps_pool.tile([P, CH], F32, name=f"psq_{j}_{qh}", tag="ps")
            for cb in range(NCB):
                nc.tensor.matmul(
                    ps[:],
                    lhsT=qwT_sb[:, cb, qh * P : (qh + 1) * P],
                    rhs=xts[cb][:],
                    start=(cb == 0),
                    stop=(cb == NCB - 1),
                )
            # Scalar-engine evac: frees DVE (mask-adds) and unblocks the
            # PSUM pool faster so downstream matmuls don't stall on buffers.
            nc.scalar.copy(QT_sb[:, qh, t0 : t0 + CH], ps[:])

        def project_k(j, xts):
            t0 = j * CH
            psk = ps_pool.tile([P, CH], F32, name=f"psk_{j}", tag="ps")
            for cb in range(NCB):
                nc.tensor.matmul(
                    psk[:],
                    lhsT=kwT_sb[:, cb, :],
                    rhs=xts[cb][:],
                    start=(cb == 0),
                    stop=(cb == NCB - 1),
                )
            nc.scalar.copy(KT_sb[:, t0 : t0 + CH], psk[:])

        def project_v(j, xts):
            t0 = j * CH
            for tb in range(CH // P):
                gtb = j * (CH // P) + tb
                psv = ps_pool.tile([P, P], F32, name=f"psv_{j}_{tb}", tag="ps")
                for cb in range(NCB):
                    nc.tensor.matmul(
                        psv[:],
                        lhsT=xts[cb][:, tb * P : (tb + 1) * P],
                        rhs=vwT_sb[:, cb, :],
                        start=(cb == 0),
                        stop=(cb == NCB - 1),
                    )
                nc.vector.tensor_copy(Vaug_sb[:, gtb, 0:128], psv[:])

        def oproj_tblock(tb):
            for nch in range(C // CH):
                pso = ps_pool.tile([P, CH], F32, name=f"pso_{tb}_{nch}", tag="ps")
                for hb in range(NQH):
                    nc.tensor.matmul(
                        pso[:],
                        lhsT=yT_sb[:, hb, tb * P : (tb + 1) * P],
                        rhs=owT_sb[:, hb, nch * CH : (nch + 1) * CH],
                        start=(hb == 0),
                        stop=(hb == NQH - 1),
                    )
                ot = oev_pool.tile([P, CH], F16, name=f"ot_{tb}_{nch}", tag="ot")
                if (tb + nch) % 2 == 0:
                    nc.scalar.copy(ot[:], pso[:])
                else:
                    nc.vector.tensor_copy(ot[:], pso[:])
                nc.sync.dma_start(
                    out_ap[tb * P : (tb + 1) * P, nch * CH : (nch + 1) * CH], ot[:]
                )

        def attention_chunk(j, pending_tbs=(), fillers=()):
            q0 = j * CH
            nkb = 4 * j + 4
            pending_tbs = list(pending_tbs)
            fillers = list(fillers)
            # ALiBi band limit for local head slot 0 (the narrowest-slope head
            # of every group is ordered first: h0/h4/h8/h12, slopes >= 0.011).
            # Blocks with q-k >= 11*128 contribute < exp(-14) of softmax mass;
            # skip their scores/AV entirely.
            kb_lo_of = lambda h: max(0, 4 * j - 7) if h == 0 else 0

            def emit_scores(h):
                """Emit score matmul + mask-add + exp for one kb block; a
                generator so AV work of the previous head can be staggered
                between score blocks (keeps the in-order PE queue from
                stalling on the DVE->ACT softmax chain)."""
                ets = {}
                for i, kb in enumerate(range(kb_lo_of(h), nkb)):
                    oi = kb - 4 * j
                    # q-columns below oi*P are fully causal-masked; skip them
                    qoff = oi * P if oi > 0 else 0
                    pss = ps_pool.tile([P, CH], F32, name=f"pss_{h}_{j}_{kb}", tag="ps")
                    nc.tensor.matmul(
                        pss[:, qoff:],
                        lhsT=KT_sb[:, kb * P : (kb + 1) * P],
                        rhs=QT_sb[:, h, q0 + qoff : q0 + CH],
                        start=True,
                        stop=True,
                    )
                    t1 = t1_pool.tile([P, CH], F16, name=f"t1_{h}_{j}_{kb}", tag="t1")
                    oidx = oi + 12
                    # t1 = pss*(SCALE/slope) + (k-q); exp(slope*t1) below.
                    # (offloading adds to GPSIMD fails in walrus codegen:
                    # GPSIMD cannot read PSUM operands here)
                    nc.vector.scalar_tensor_tensor(
                        t1[:, qoff:],
                        pss[:, qoff:],
                        sc_sb[:, h, 0:1],
                        m_sb[:, oidx, qoff:],
                        op0=mybir.AluOpType.mult,
                        op1=mybir.AluOpType.add,
                    )
                    et = exp_pool.tile([P, CH], F16, name=f"et_{h}_{j}_{kb}", tag="et")
                    nc.scalar.activation(
                        et[:, qoff:],
                        t1[:, qoff:],
                        EXP,
                        scale=sc_sb[:, h, 1:2],
                    )
                    ets[kb] = et
                    yield ets

            def make_av_emitter(h):
                """Per-qb AV chain + delayed-normalize + transpose, with the
                PE transpose of block qb staggered behind AV of block qb+1 so
                it never heads the PE queue while its DVE inputs are pending."""
                pend = []

                def flush(n):
                    while len(pend) > n:
                        gqb, yps = pend.pop(0)
                        recip = rc_pool.tile([P, 1], F32, name=f"rc_{h}_{gqb}", tag="rc")
                        nc.vector.reciprocal(recip[:], yps[:, 128:129])
                        yn = yn_pool.tile([P, P], F16, name=f"yn_{h}_{gqb}", tag="yn")
                        nc.vector.tensor_scalar_mul(yn[:], yps[:, 0:128], recip[:])
                        tp = tp_pool.tile([P, P], F16, name=f"tp_{h}_{gqb}", tag="tp")
                        nc.tensor.transpose(tp[:], yn[:], id_sb[:])
                        nc.vector.tensor_copy(
                            yT_sb[:, h, gqb * P : (gqb + 1) * P], tp[:]
                        )

                def emit_one(qb, ets):
                    gqb = j * (CH // P) + qb
                    lo = kb_lo_of(h)
                    yps = yps_pool.tile([P, 132], F32, name=f"yps_{h}_{gqb}", tag="yps")
                    for kb in range(lo, gqb + 1):
                        nc.tensor.matmul(
                            yps[:, 0:129],
                            lhsT=ets[kb][:, qb * P : (qb + 1) * P],
                            rhs=Vaug_sb[:, kb, 0:129],
                            start=(kb == lo),
                            stop=(kb == gqb),
                        )
                    pend.append((gqb, yps))
                    flush(1)

                return emit_one, flush

            # NOTE: interleaving AV chains into the score loop measures ~5us
            # WORSE: a short 129-stream before a fat score matmul exposes the
            # fat matmul's weight load (LDW conservation) — keep smalls grouped.
            # DVE (mask-adds) and ACT (exps) saturate inside attention spans
            # while the PE has spare capacity, so previous-chunk o-projection
            # t-blocks are spread one-per-head here: independent fat PE work
            # that keeps the PE busy while the softmax chain drains.
            # Cross-head software pipeline: while head h's AV chain runs
            # (short dependent matmuls), head h+1's fat score matmuls are
            # interleaved so the in-order PE queue always has independent
            # work while the DVE->ACT softmax chain of h+1 drains.
            gens = [emit_scores(h) for h in range(NQH)]
            etss = [None] * NQH
            for etss[0] in gens[0]:
                pass
            nsc = nkb  # score blocks to advance per head (>= emitted count)
            for h in range(NQH):
                emit_one, flush = make_av_emitter(h)
                nxt = gens[h + 1] if h + 1 < NQH else None
                step = (nsc + CH // P - 1) // (CH // P)
                last = (j == NCHK - 1) and (h == NQH - 1)
                for qb in range(CH // P):
                    emit_one(qb, etss[h])
                    if nxt is not None:
                        for _ in range(step):
                            try:
                                etss[h + 1] = next(nxt)
                            except StopIteration:
                                break
                    if last:
                        if qb == 0:
                            if pending_tbs:
                                oproj_tblock(pending_tbs.pop(0))
                        else:
                            oproj_tblock(j * (CH // P) + qb - 1)
                if nxt is not None:
                    for etss[h + 1] in nxt:
                        pass
                if not last:
                    if pending_tbs:
                        oproj_tblock(pending_tbs.pop(0))
                    if fillers:
                        fillers.pop(0)()
                flush(0)
                if last:
                    oproj_tblock(j * (CH // P) + (CH // P) - 1)
            for tb in pending_tbs:
                oproj_tblock(tb)
            for f in fillers:
                f()

        # ---- prologue: chunk-0 projections, DMA-order-tuned ----
        xts_cur = project_x(0)
        nc.sync.dma_start(kwT_sb[:], kwT_ap[:])
        nc.sync.dma_start(sc_sb[:], sc_ap[:])
        # chunk 0's blocks are all diagonal (oi >= 0): that mask slice plus
        # sc is only 0.5MB -- land it before the fat weight tail so
        # attention(0) never waits ~20us on the full mask tensor.
        nc.sync.dma_start(m_sb[:, 12:16, :], m_ap[:, 12:16, :])
        nc.sync.dma_start(vwT_sb[:], vwT_ap[:])
        nc.sync.dma_start(qwT_sb[:, :, P:], qwT_ap[:, :, P:])
        qproj_unit(0, 0, xts_cur)
        project_k(0, xts_cur)
        project_v(0, xts_cur)
        for qh in range(1, NQH):
            qproj_unit(0, qh, xts_cur)
        # mask/bias tiles next: attention chunk 0 stalls on them well before
        # the x prefetch of chunk 1 or owT are needed.
        nc.sync.dma_start(m_sb[:, 0:12, :], m_ap[:, 0:12, :])

        for j in range(NCHK):
            if j + 1 < NCHK:
                xts_next = project_x(j + 1)
                fillers = [
                    (lambda jj=j + 1, qq=qh, xx=xts_next: qproj_unit(jj, qq, xx))
                    for qh in range(NQH)
                ]
            else:
                xts_next = None
                fillers = []
            if j == 0:
                nc.sync.dma_start(owT_sb[:], owT_ap[:])
            # o-projection t-blocks of chunk j-1 are spread inside
            # attention(j), one per head (see attention_chunk).
            prev_tbs = range((j - 1) * (CH // P), j * (CH // P)) if j > 0 else ()
            attention_chunk(j, prev_tbs, fillers=fillers)
            if xts_next is not None:
                project_k(j + 1, xts_next)
                project_v(j + 1, xts_next)

    nc.compile()
    return nc


def make_in_maps(x, q_w, k_w, v_w, o_w):
    """Host-side sharding/preprocessing -> per-core input dicts."""
    slopes = _alibi_slopes(H)
    x_bf = np.asarray(x, dtype=NP_F16)

    ident = np.eye(P, dtype=NP_F16)

    pi = np.arange(P, dtype=np.float32)[:, None]
    mj = np.arange(CH, dtype=np.float32)[None, :]

    # Relative-position mask, shared by all heads/cores: exact small ints
    # in f16 (|k-q| <= 2047 < 2048), causal entries -> -60000 (underflows
    # the f16 exp for every slope).
    mrel = np.empty((P, 16, CH), dtype=np.float32)
    for oidx in range(16):
        X = (oidx - 12) * P + pi - mj
        mrel[:, oidx, :] = np.where(X > 0.0, np.float32(MASK_NEG), X)
    mrel = mrel.astype(np.float16)

    in_maps = []
    for c in range(8):
        b, g = c // 4, c % 4
        qsl = slice(4 * g * P, (4 * g + 4) * P)
        ksl = slice(g * P, (g + 1) * P)

        qwT = np.ascontiguousarray(
            np.asarray(q_w[qsl].T, dtype=NP_F16).reshape(NCB, P, NQH * P).transpose(1, 0, 2)
        )
        kwT = np.ascontiguousarray(
            np.asarray(k_w[ksl].T, dtype=NP_F16).reshape(NCB, P, D).transpose(1, 0, 2)
        )
        vwT = np.ascontiguousarray(
            np.asarray(v_w[ksl].T, dtype=NP_F16).reshape(NCB, P, D).transpose(1, 0, 2)
        )
        owT = np.ascontiguousarray(
            np.asarray(o_w[:, qsl].T, dtype=NP_F16).reshape(NQH, P, C).transpose(1, 0, 2)
        )

        sctbl = np.empty((P, NQH, 2), dtype=np.float32)
        for h in range(NQH):
            sl = np.float64(slopes[4 * g + h])
            sctbl[:, h, 0] = np.float32(SCALE / sl)
            sctbl[:, h, 1] = np.float32(sl)

        in_maps.append(
            dict(
                xT=np.ascontiguousarray(x_bf[b].T),
                qwT=qwT,
                kwT=kwT,
                vwT=vwT,
                owT=owT,
                mrel=mrel,
                sctbl=sctbl,
                ident=ident,
            )
        )
    return in_maps


def gather_output(results):
    out = np.zeros((B, T, C), dtype=np.float32)
    for c in range(8):
        out[c // 4] += results[c]["out_p"].astype(np.float32)
    return out


_NC_CACHE = {}


def get_program():
    if "nc" not in _NC_CACHE:
        _NC_CACHE["nc"] = build_program()
    return _NC_CACHE["nc"]


def kernel(x, q_w, k_w, v_w, o_w):
    from concourse.bass_utils import run_bass_kernel_spmd

    nc = get_program()
    in_maps = make_in_maps(x, q_w, k_w, v_w, o_w)
    res = run_bass_kernel_spmd(nc, in_maps, list(range(8)))
    return gather_output(res.results)



# revision 26
# speedup vs baseline: 1.2227x; 1.0111x over previous
"""Trainium2 Bass kernel for causal self-attention with ALiBi + GQA.

Problem: B=2, T=2048, C=2048, 16 q-heads / 4 kv-heads, head_dim=128.
  q = x@q_w.T, k = x@k_w.T, v = x@v_w.T (GQA repeat 4x)
  att = softmax(q k^T/sqrt(d) + causal + alibi); out = (att v) @ o_w.T

Sharding over 8 NeuronCores: core c -> batch c//4, kv-group g=c%4
(q-heads 4g..4g+3, kv-head g).  Each core computes attention for its 4
heads on its batch plus a partial o-projection over its 512 channels;
the host sums the 4 partials per batch.

On-chip design (per core, all matmuls fp16, fp32 accumulate — fp16 is
1 cycle/row on the PE like bf16 but with 4x the mantissa; all values
here are bounded so there is no range risk):
  - x is host-cast to fp16 and host-TRANSPOSED (xT [C,T]) so projection
    moving operands load with plain contiguous DMA.
  - Projections make QT [d,t], KT [d,t] (transposed) and V natural
    [t,d] with a ones-column appended, so the AV matmul also emits the
    softmax denominator for free.
  - Scores are computed transposed sT[k,q] = KTblk.T @ QT (moving free
    dim 512, causally narrowed per diagonal offset); ALiBi + causal
    folded in via host-precomputed additive f32 tiles (DVE) and a
    per-(head,offset) bias in the ACT exp.  No max-subtraction needed:
    scores are small (~N(0,0.8)) and masked entries use -1e9.
  - ALiBi+causal masking uses ONE head-independent relative-position
    table M[pi,oidx,mj] = k-q (exact small ints in f16; causal ->
    -60000) plus per-core [P,1] scalar tables: DVE fused
    scalar_tensor_tensor computes t1 = pss*(SCALE/slope) + (k-q) in
    f16, ACT exp applies scale=slope from an AP.  Storing k-q directly
    (instead of slope-scaled masks) keeps f16 rounding error ~5e-4 of
    the EXPONENT at the entries that matter -- slope-scaled f16 masks
    lose 1e-2 to large-term cancellation.  Far blocks underflow exp to
    0, which is exactly the negligible tail of the softmax.
  - ALiBi band sparsity: the narrowest-band head of each group is
    ordered first (h0/h4/h8/h12, slopes >= 0.011); its score/AV blocks
    with q-k >= 11*128 (softmax mass < e^-14) are skipped.
  - y accumulates un-normalized; delayed normalization via per-row
    reciprocal of the ones-column sums, then PE-transpose -> yT feeds
    the o-projection (psum -> ACT/DVE copy -> fp16 -> DMA out; host
    sums the 4 partials per batch in fp32).
  - Scheduling: a warmup matmul burst ramps the PE p-state; chunk
    j+1's x tiles are prefetched and its per-head Q-projection units
    emitted at attention(j) head boundaries (with prev-chunk
    o-projection t-blocks) BEFORE the yT flush, so the in-order PE
    queue always has fat independent work while the DVE->ACT softmax
    chain drains; head h+1's score matmuls are software-pipelined
    into head h's AV chain; the last chunk's o-projection t-blocks
    interleave into its last head's AV loop (no serial tail); chunk
    0's diagonal mask slice (0.5MB) is DMA'd before the fat weight
    tail so attention(0) never waits on the full 2MB mask tensor.
Measured: 265.1-268 us/core on TRN2 (NTFF; occasional +15-50us
outlier runs from device contention -- compare configs by min-of-3),
L2 rel err 6.6e-4 vs fp32 reference.  (Session baseline: 272-274us.)
PE busy ~231us of a ~202us pure-row floor; the residual is small-matmul
LDW exposure in the AV stage (544 stationary loads, structural) plus
~20us of softmax-chain stalls.  Tried and reverted this session: fat
V-proj via PE transposes (tp-pool head-of-line blocking causes chip-
wide p-state drops), batched strided xT DMA (descriptor inefficiency,
13.8us transfers), const DMAs on the ACT HWDGE queue (2MB owT stalls
the ACT compute queue), K-proj as an attention filler + no chunk-0
fillers (net loss), yps=3/ps=4 PSUM rebalance (scores pipeline
throttles on 4 psum bufs).  fp8 in any stage fails the 2e-2 gate
(measured 3.5-8e-2 on host); cross-core K/V sharing for full band-
sparsity rebalancing needs collectives whose ~45-60us latency
(measured via a DRAM AllGather probe) does not pipeline at this
kernel's scale.

Rejected directions (measured on this HW): AV with V-stationary (fat
512-wide moving) needs softmax denominators off-PE — GPSIMD add-chains
+ partition_all_reduce are ~4x slower than the cost model (634us total);
XBAR dma_start_transpose from SBUF (direct or DRAM-bounced) returns
wrong data (NaN) in this axon environment; matmul moving free dim is
hard-capped at 512 so wider PSUM tiles are impossible. The ones-column
AV is LDW-conservation-optimal: any orientation computing denominators
on-PE needs the same 544 stationary loads.
"""

import math
import sys
from contextlib import ExitStack

import numpy as np

sys.path.insert(0, "/opt/trn_rl_repo")

import ml_dtypes  # noqa: E402

import concourse.bacc as bacc  # noqa: E402
import concourse.bass as bass  # noqa: E402
import concourse.mybir as mybir  # noqa: E402
import concourse.tile as tile  # noqa: E402

F16 = mybir.dt.float16
F32 = mybir.dt.float32
NP_F16 = np.float16

B, T, C = 2, 2048, 2048
H, HKV, D = 16, 4, 128
P = 128
CH = 512                 # q-chunk (moving free dim)
NCB = C // P             # 16 contraction blocks
NTB = T // P             # 16 t-blocks
NCHK = T // CH           # 4 q-chunks
NQH = 4                  # local q heads per core
SCALE = 1.0 / math.sqrt(D)
MASK_NEG = -60000.0     # f16-representable; exp(scale*(-60000)) == 0


def _alibi_slopes(n):
    start = 2 ** (-(2 ** (-(math.log2(n) - 3))))
    return np.array([start * start**i for i in range(n)], dtype=np.float64)


def build_program():
    """Build the (SPMD-identical) single-core program."""
    nc = bacc.Bacc("TRN2", target_bir_lowering=False, debug=False, num_devices=8)

    xT_ap = nc.dram_tensor("xT", [C, T], F16, kind="ExternalInput").ap()
    qwT_ap = nc.dram_tensor("qwT", [P, NCB, NQH * P], F16, kind="ExternalInput").ap()
    kwT_ap = nc.dram_tensor("kwT", [P, NCB, D], F16, kind="ExternalInput").ap()
    vwT_ap = nc.dram_tensor("vwT", [P, NCB, D], F16, kind="ExternalInput").ap()
    owT_ap = nc.dram_tensor("owT", [P, NQH, C], F16, kind="ExternalInput").ap()
    # M[pi, oidx, mj] = k - q (exact small ints in f16) with causal -60000;
    # head-independent: the slope is applied via per-core scalar tables.
    m_ap = nc.dram_tensor("mrel", [P, 16, CH], F16, kind="ExternalInput").ap()
    sc_ap = nc.dram_tensor("sctbl", [P, NQH, 2], F32, kind="ExternalInput").ap()
    id_ap = nc.dram_tensor("ident", [P, P], F16, kind="ExternalInput").ap()
    out_ap = nc.dram_tensor("out_p", [T, C], F16, kind="ExternalOutput").ap()

    EXP = mybir.ActivationFunctionType.Exp

    with tile.TileContext(nc) as tc, ExitStack() as ctx:
        const = ctx.enter_context(tc.tile_pool(name="const", bufs=1))
        qwT_sb = const.tile([P, NCB, NQH * P], F16, name="qwT_sb")
        kwT_sb = const.tile([P, NCB, D], F16, name="kwT_sb")
        vwT_sb = const.tile([P, NCB, D], F16, name="vwT_sb")
        owT_sb = const.tile([P, NQH, C], F16, name="owT_sb")
        m_sb = const.tile([P, 16, CH], F16, name="m_sb")
        sc_sb = const.tile([P, NQH, 2], F32, name="sc_sb")
        id_sb = const.tile([P, P], F16, name="id_sb")

        wid_sb = const.tile([P, P], F16, name="wid_sb")
        QT_sb = const.tile([P, NQH, T], F16, name="QT_sb")
        KT_sb = const.tile([P, T], F16, name="KT_sb")
        Vaug_sb = const.tile([P, NTB, 132], F16, name="Vaug_sb")
        yT_sb = const.tile([P, NQH, T], F16, name="yT_sb")

        # DMA order tuned so warmup + the first productive matmul (Q-proj
        # head 0 of chunk 0) start as early as possible: tiny identity
        # first (warmup reads it), qwT head-0 slice next, then the chunk-0
        # x tiles (issued by project_x below), then the rest.
        nc.sync.dma_start(id_sb[:], id_ap[:])
        nc.sync.dma_start(qwT_sb[:, :, 0:P], qwT_ap[:, :, 0:P])

        nc.vector.memset(Vaug_sb[:, :, 128:129], 1.0)

        xT_pool = ctx.enter_context(tc.tile_pool(name="xT_pool", bufs=32))
        ps_pool = ctx.enter_context(tc.tile_pool(name="ps_pool", bufs=5, space="PSUM"))
        yps_pool = ctx.enter_context(tc.tile_pool(name="yps_pool", bufs=2, space="PSUM"))
        tp_pool = ctx.enter_context(tc.tile_pool(name="tp_pool", bufs=1, space="PSUM"))
        t1_pool = ctx.enter_context(tc.tile_pool(name="t1_pool", bufs=8))
        exp_pool = ctx.enter_context(tc.tile_pool(name="exp_pool", bufs=34))
        oev_pool = ctx.enter_context(tc.tile_pool(name="oev_pool", bufs=8))
        yn_pool = ctx.enter_context(tc.tile_pool(name="yn_pool", bufs=6))
        rc_pool = ctx.enter_context(tc.tile_pool(name="rc_pool", bufs=6))

        # PE p-state warm-up: dependency-free dummy matmuls ramp the tensor
        # engine to full clock while the first input DMAs land.
        nc.vector.memset(wid_sb[:], 0.0)
        for w in range(100):
            wups = yps_pool.tile([P, 132], F32, name=f"wups_{w}", tag="yps")
            nc.tensor.matmul(
                wups[:, 0:128], lhsT=wid_sb[:], rhs=wid_sb[:], start=True, stop=True
            )

        # ---- Fused per-chunk pipeline: project chunk j, then attention for
        # chunk j (legal because causality means queries in chunk j only
        # attend to keys/values t <= chunk j), then its o-projection.
        # Chunk j+1's x tiles are prefetched and its per-head Q-projection
        # units emitted INSIDE attention(j) at head boundaries: independent
        # fat PE work that absorbs the DVE/ACT softmax-chain latency. ----
        def project_x(j):
            t0 = j * CH
            xts = []
            for cb in range(NCB):
                xt = xT_pool.tile([P, CH], F16, name=f"xt_{j}_{cb}", tag="xt")
                nc.sync.dma_start(xt[:], xT_ap[cb * P : (cb + 1) * P, t0 : t0 + CH])
                xts.append(xt)
            return xts

        def qproj_unit(j, qh, xts):
            t0 = j * CH
            ps = ps_pool.tile([P, CH], F32, name=f"psq_{j}_{qh}", tag="ps")
            for cb in range(NCB):
                nc.tensor.matmul(
                    ps[:],
                    lhsT=qwT_sb[:, cb, qh * P : (qh + 1) * P],
                    rhs=xts[cb][:],
                    start=(cb == 0),
                    stop=(cb == NCB - 1),
                )
            # Scalar-engine evac: frees DVE (mask-adds) and unblocks the
            # PSUM pool faster so downstream matmuls don't stall on buffers.
            nc.scalar.copy(QT_sb[:, qh, t0 : t0 + CH], ps[:])

        def project_k(j, xts):
            t0 = j * CH
            psk = ps_pool.tile([P, CH], F32, name=f"psk_{j}", tag="ps")
            for cb in range(NCB):
                nc.tensor.matmul(
                    psk[:],
                    lhsT=kwT_sb[:, cb, :],
                    rhs=xts[cb][:],
                    start=(cb == 0),
                    stop=(cb == NCB - 1),
                )
            nc.scalar.copy(KT_sb[:, t0 : t0 + CH], psk[:])

        def project_v(j, xts):
            t0 = j * CH
            for tb in range(CH // P):
                gtb = j * (CH // P) + tb
                psv = ps_pool.tile([P, P], F32, name=f"psv_{j}_{tb}", tag="ps")
                for cb in range(NCB):
                    nc.tensor.matmul(
                        psv[:],
                        lhsT=xts[cb][:, tb * P : (tb + 1) * P],
                        rhs=vwT_sb[:, cb, :],
                        start=(cb == 0),
                        stop=(cb == NCB - 1),
                    )
                nc.vector.tensor_copy(Vaug_sb[:, gtb, 0:128], psv[:])

        def oproj_tblock(tb):
            for nch in range(C // CH):
                pso = ps_pool.tile([P, CH], F32, name=f"pso_{tb}_{nch}", tag="ps")
                for hb in range(NQH):
                    nc.tensor.matmul(
                        pso[:],
                        lhsT=yT_sb[:, hb, tb * P : (tb + 1) * P],
                        rhs=owT_sb[:, hb, nch * CH : (nch + 1) * CH],
                        start=(hb == 0),
                        stop=(hb == NQH - 1),
                    )
                ot = oev_pool.tile([P, CH], F16, name=f"ot_{tb}_{nch}", tag="ot")
                if (tb + nch) % 2 == 0:
                    nc.scalar.copy(ot[:], pso[:])
                else:
                    nc.vector.tensor_copy(ot[:], pso[:])
                nc.sync.dma_start(
                    out_ap[tb * P : (tb + 1) * P, nch * CH : (nch + 1) * CH], ot[:]
                )

        def attention_chunk(j, pending_tbs=(), fillers=()):
            q0 = j * CH
            nkb = 4 * j + 4
            pending_tbs = list(pending_tbs)
            fillers = list(fillers)
            # ALiBi band limit for local head slot 0 (the narrowest-slope head
            # of every group is ordered first: h0/h4/h8/h12, slopes >= 0.011).
            # Blocks with q-k >= 11*128 contribute < exp(-14) of softmax mass;
            # skip their scores/AV entirely.
            kb_lo_of = lambda h: max(0, 4 * j - 7) if h == 0 else 0

            def emit_scores(h):
                """Emit score matmul + mask-add + exp for one kb block; a
                generator so AV work of the previous head can be staggered
                between score blocks (keeps the in-order PE queue from
                stalling on the DVE->ACT softmax chain)."""
                ets = {}
                for i, kb in enumerate(range(kb_lo_of(h), nkb)):
                    oi = kb - 4 * j
                    # q-columns below oi*P are fully causal-masked; skip them
                    qoff = oi * P if oi > 0 else 0
                    pss = ps_pool.tile([P, CH], F32, name=f"pss_{h}_{j}_{kb}", tag="ps")
                    nc.tensor.matmul(
                        pss[:, qoff:],
                        lhsT=KT_sb[:, kb * P : (kb + 1) * P],
                        rhs=QT_sb[:, h, q0 + qoff : q0 + CH],
                        start=True,
                        stop=True,
                    )
                    t1 = t1_pool.tile([P, CH], F16, name=f"t1_{h}_{j}_{kb}", tag="t1")
                    oidx = oi + 12
                    # t1 = pss*(SCALE/slope) + (k-q); exp(slope*t1) below.
                    # (offloading adds to GPSIMD fails in walrus codegen:
                    # GPSIMD cannot read PSUM operands here)
                    nc.vector.scalar_tensor_tensor(
                        t1[:, qoff:],
                        pss[:, qoff:],
                        sc_sb[:, h, 0:1],
                        m_sb[:, oidx, qoff:],
                        op0=mybir.AluOpType.mult,
                        op1=mybir.AluOpType.add,
                    )
                    et = exp_pool.tile([P, CH], F16, name=f"et_{h}_{j}_{kb}", tag="et")
                    nc.scalar.activation(
                        et[:, qoff:],
                        t1[:, qoff:],
                        EXP,
                        scale=sc_sb[:, h, 1:2],
                    )
                    ets[kb] = et
                    yield ets

            def make_av_emitter(h):
                """Per-qb AV chain + delayed-normalize + transpose, with the
                PE transpose of block qb staggered behind AV of block qb+1 so
                it never heads the PE queue while its DVE inputs are pending."""
                pend = []

                def flush(n):
                    while len(pend) > n:
                        gqb, yps = pend.pop(0)
                        recip = rc_pool.tile([P, 1], F32, name=f"rc_{h}_{gqb}", tag="rc")
                        nc.vector.reciprocal(recip[:], yps[:, 128:129])
                        yn = yn_pool.tile([P, P], F16, name=f"yn_{h}_{gqb}", tag="yn")
                        nc.vector.tensor_scalar_mul(yn[:], yps[:, 0:128], recip[:])
                        tp = tp_pool.tile([P, P], F16, name=f"tp_{h}_{gqb}", tag="tp")
                        nc.tensor.transpose(tp[:], yn[:], id_sb[:])
                        nc.vector.tensor_copy(
                            yT_sb[:, h, gqb * P : (gqb + 1) * P], tp[:]
                        )

                def emit_one(qb, ets):
                    gqb = j * (CH // P) + qb
                    lo = kb_lo_of(h)
                    yps = yps_pool.tile([P, 132], F32, name=f"yps_{h}_{gqb}", tag="yps")
                    for kb in range(lo, gqb + 1):
                        nc.tensor.matmul(
                            yps[:, 0:129],
                            lhsT=ets[kb][:, qb * P : (qb + 1) * P],
                            rhs=Vaug_sb[:, kb, 0:129],
                            start=(kb == lo),
                            stop=(kb == gqb),
                        )
                    pend.append((gqb, yps))
                    flush(1)

                return emit_one, flush

            # NOTE: interleaving AV chains into the score loop measures ~5us
            # WORSE: a short 129-stream before a fat score matmul exposes the
            # fat matmul's weight load (LDW conservation) — keep smalls grouped.
            # DVE (mask-adds) and ACT (exps) saturate inside attention spans
            # while the PE has spare capacity, so previous-chunk o-projection
            # t-blocks are spread one-per-head here: independent fat PE work
            # that keeps the PE busy while the softmax chain drains.
            # Cross-head software pipeline: while head h's AV chain runs
            # (short dependent matmuls), head h+1's fat score matmuls are
            # interleaved so the in-order PE queue always has independent
            # work while the DVE->ACT softmax chain of h+1 drains.
            gens = [emit_scores(h) for h in range(NQH)]
            etss = [None] * NQH
            for etss[0] in gens[0]:
                pass
            nsc = nkb  # score blocks to advance per head (>= emitted count)
            for h in range(NQH):
                emit_one, flush = make_av_emitter(h)
                nxt = gens[h + 1] if h + 1 < NQH else None
                step = (nsc + CH // P - 1) // (CH // P)
                last = (j == NCHK - 1) and (h == NQH - 1)
                for qb in range(CH // P):
                    emit_one(qb, etss[h])
                    if nxt is not None:
                        for _ in range(step):
                            try:
                                etss[h + 1] = next(nxt)
                            except StopIteration:
                                break
                    if last:
                        if qb == 0:
                            if pending_tbs:
                                oproj_tblock(pending_tbs.pop(0))
                        else:
                            oproj_tblock(j * (CH // P) + qb - 1)
                if nxt is not None:
                    for etss[h + 1] in nxt:
                        pass
                if not last:
                    if pending_tbs:
                        oproj_tblock(pending_tbs.pop(0))
                    if fillers:
                        fillers.pop(0)()
                flush(0)
                if last:
                    oproj_tblock(j * (CH // P) + (CH // P) - 1)
            for tb in pending_tbs:
                oproj_tblock(tb)
            for f in fillers:
                f()

        # ---- prologue: chunk-0 projections, DMA-order-tuned ----
        xts_cur = project_x(0)
        nc.sync.dma_start(kwT_sb[:], kwT_ap[:])
        nc.sync.dma_start(sc_sb[:], sc_ap[:])
        # chunk 0's blocks are all diagonal (oi >= 0): that mask slice plus
        # sc is only 0.5MB -- land it before the fat weight tail so
        # attention(0) never waits ~20us on the full mask tensor.
        nc.sync.dma_start(m_sb[:, 12:16, :], m_ap[:, 12:16, :])
        nc.sync.dma_start(vwT_sb[:], vwT_ap[:])
        nc.sync.dma_start(qwT_sb[:, :, P:], qwT_ap[:, :, P:])
        qproj_unit(0, 0, xts_cur)
        project_k(0, xts_cur)
        project_v(0, xts_cur)
        for qh in range(1, NQH):
            qproj_unit(0, qh, xts_cur)
        # mask/bias tiles next: attention chunk 0 stalls on them well before
        # the x prefetch of chunk 1 or owT are needed.
        nc.sync.dma_start(m_sb[:, 0:12, :], m_ap[:, 0:12, :])

        for j in range(NCHK):
            if j + 1 < NCHK:
                xts_next = project_x(j + 1)
                fillers = [
                    (lambda jj=j + 1, qq=qh, xx=xts_next: qproj_unit(jj, qq, xx))
                    for qh in range(NQH)
                ]
            else:
                xts_next = None
                fillers = []
            if j == 0:
                nc.sync.dma_start(owT_sb[:], owT_ap[:])
            # o-projection t-blocks of chunk j-1 are spread inside
            # attention(j), one per head (see attention_chunk).
            prev_tbs = range((j - 1) * (CH // P), j * (CH // P)) if j > 0 else ()
            attention_chunk(j, prev_tbs, fillers=fillers)
            if xts_next is not None:
                project_k(j + 1, xts_next)
                project_v(j + 1, xts_next)

    nc.compile()
    return nc


def make_in_maps(x, q_w, k_w, v_w, o_w):
    """Host-side sharding/preprocessing -> per-core input dicts."""
    slopes = _alibi_slopes(H)
    x_bf = np.asarray(x, dtype=NP_F16)

    ident = np.eye(P, dtype=NP_F16)

    pi = np.arange(P, dtype=np.float32)[:, None]
    mj = np.arange(CH, dtype=np.float32)[None, :]

    # Relative-position mask, shared by all heads/cores: exact small ints
    # in f16 (|k-q| <= 2047 < 2048), causal entries -> -60000 (underflows
    # the f16 exp for every slope).
    mrel = np.empty((P, 16, CH), dtype=np.float32)
    for oidx in range(16):
        X = (oidx - 12) * P + pi - mj
        mrel[:, oidx, :] = np.where(X > 0.0, np.float32(MASK_NEG), X)
    mrel = mrel.astype(np.float16)

    in_maps = []
    for c in range(8):
        b, g = c // 4, c % 4
        qsl = slice(4 * g * P, (4 * g + 4) * P)
        ksl = slice(g * P, (g + 1) * P)

        qwT = np.ascontiguousarray(
            np.asarray(q_w[qsl].T, dtype=NP_F16).reshape(NCB, P, NQH * P).transpose(1, 0, 2)
        )
        kwT = np.ascontiguousarray(
            np.asarray(k_w[ksl].T, dtype=NP_F16).reshape(NCB, P, D).transpose(1, 0, 2)
        )
        vwT = np.ascontiguousarray(
            np.asarray(v_w[ksl].T, dtype=NP_F16).reshape(NCB, P, D).transpose(1, 0, 2)
        )
        owT = np.ascontiguousarray(
            np.asarray(o_w[:, qsl].T, dtype=NP_F16).reshape(NQH, P, C).transpose(1, 0, 2)
        )

        sctbl = np.empty((P, NQH, 2), dtype=np.float32)
        for h in range(NQH):
            sl = np.float64(slopes[4 * g + h])
            sctbl[:, h, 0] = np.float32(SCALE / sl)
            sctbl[:, h, 1] = np.float32(sl)

        in_maps.append(
            dict(
                xT=np.ascontiguousarray(x_bf[b].T),
                qwT=qwT,
                kwT=kwT,
                vwT=vwT,
                owT=owT,
                mrel=mrel,
                sctbl=sctbl,
                ident=ident,
            )
        )
    return in_maps


def gather_output(results):
    out = np.zeros((B, T, C), dtype=np.float32)
    for c in range(8):
        out[c // 4] += results[c]["out_p"].astype(np.float32)
    return out


_NC_CACHE = {}


def get_program():
    if "nc" not in _NC_CACHE:
        _NC_CACHE["nc"] = build_program()
    return _NC_CACHE["nc"]


def kernel(x, q_w, k_w, v_w, o_w):
    from concourse.bass_utils import run_bass_kernel_spmd

    nc = get_program()
    in_maps = make_in_maps(x, q_w, k_w, v_w, o_w)
    res = run_bass_kernel_spmd(nc, in_maps, list(range(8)))
    return gather_output(res.results)

